# revision 2
# baseline (speedup 1.0000x reference)
"""HardMaxAttention Trainium2 Bass kernel, v4.

Like v3 (chunked pairwise collectives, supergroup-interleaved key/V
layout, masks folded into PSUM, scalar PSUM->SBUF copies) plus:

  * scores run as ONE bf16 matmul with K=8 instead of fp32 LOW_HIGH
    (which streams every column twice).  Q^T/K^T (fp32, from phase 1)
    are split on-device into bf16 hi+lo rows; with lhsT rows
    [qh,ql,qh,ql] and rhs rows [kh,kh,kl,kl] the K=8 contraction
    computes (qh+ql)*(kh+kl) exactly (rel err ~2^-26, fp32-equivalent;
    the systolic array is 128 deep so K=8 streams at the same rate as
    K=2).  2x faster than LOW_HIGH.
  * the K AllGather payload is the bf16 hi/lo split (two chunks of 2
    phase-1 groups each, so early score tiles start sooner).
  * masks/identity in bf16 (halves mask-matmul stream time).
  * xq group 0 is DMA'd per k-chunk so phase 1 starts ~3us in.
  * V tiles paced 1 per score tile (PE stays busy through the
    vector-bound tail, keeping the HAM clock-gate at full rate).
"""

import numpy as np

B, T, D, H = 4, 4096, 1024, 2
P = 128
NT = T // P            # 32 t-tiles per batch
MYT = NT // 2          # 16 t-tiles per core
KD = D // P            # 8 contraction blocks
HALF = T // 2          # 2048 own positions per core
NGH = HALF // 512      # 4 transposed groups (= supergroup quarters)
N_CORES = 8
NEG = -1.0e30
GROUPS = [[0, 1], [2, 3], [4, 5], [6, 7]]

_prog_cache = {}


def _build_program():
    import concourse.bacc as bacc
    import concourse.mybir as mybir
    import concourse.tile as tile
    import concourse.bass as bass
    from concourse.masks import make_identity

    f32 = mybir.dt.float32
    bf16 = mybir.dt.bfloat16
    u32 = mybir.dt.uint32

    nc = bacc.Bacc(None, target_bir_lowering=False, num_devices=N_CORES)

    # xqbh[g, p, k*512+c] = bf16(x_own[g*512+c, k*128+p]); xqbl = residual
    xqbh = nc.dram_tensor("xqbh", [NGH, P, KD * 512], bf16,
                          kind="ExternalInput")
    xqbl = nc.dram_tensor("xqbl", [NGH, P, KD * 512], bf16,
                          kind="ExternalInput")
    # w_qk8 cols per k: [q0h,q1h,k0h,k1h, q0l,q1l,k0l,k1l]
    w_qk8 = nc.dram_tensor("w_qk8", [D, 8], bf16, kind="ExternalInput")
    w_vT = nc.dram_tensor("w_vT", [D, D], bf16, kind="ExternalInput")
    # amask4[r]: mask/pad block for the partial-supergroup EVEN section,
    # applied at psum cols [r*128, 512): [amask | NEG*(3-r) blocks]
    amask4 = nc.dram_tensor("amask4", [P, 4 * 512], bf16, kind="ExternalInput")
    tmask = nc.dram_tensor("tmask", [P, P], bf16, kind="ExternalInput")
    out = nc.dram_tensor("out", [MYT, P, D], bf16, kind="ExternalOutput")

    with tile.TileContext(nc) as tc:
        with (
            tc.tile_pool(name="const", bufs=1) as cpool,
            tc.tile_pool(name="xin", bufs=4) as xpool,
            tc.tile_pool(name="qk", bufs=1) as qkpool,
            tc.tile_pool(name="ktmp", bufs=2) as ktpool,
            tc.tile_pool(name="sc", bufs=3) as scpool,
            tc.tile_pool(name="small", bufs=4) as spool,
            tc.tile_pool(name="idxp", bufs=MYT) as idxpool,
            tc.tile_pool(name="vsb", bufs=3) as vpool,
            tc.tile_pool(name="xg", bufs=6) as xgpool,
            tc.tile_pool(name="qk_ps", bufs=2, space="PSUM") as qkpsum,
            tc.tile_pool(name="mm_ps", bufs=3, space="PSUM") as mmpsum,
            tc.tile_pool(name="vo_ps", bufs=2, space="PSUM") as vopsum,
            tc.tile_pool(name="dram", bufs=1, space="DRAM") as dpool,
        ):
            # ---- input loads: all on the scalar (Act) HWDGE queue so
            # the sync queue stays free for the phase-1 critical chain
            wqk_sb = cpool.tile([P, 8 * KD], bf16)
            for k in range(KD):
                nc.sync.dma_start(
                    wqk_sb[:, k * 8:(k + 1) * 8], w_qk8[k * P:(k + 1) * P, :]
                )
            xh_sb = [None] * NGH
            xl_sb = [None] * NGH
            xh_sb[0] = xpool.tile([P, KD * 512], bf16, name="xhs0", tag="xh")
            xl_sb[0] = xpool.tile([P, KD * 512], bf16, name="xls0", tag="xl")
            for k in range(0, KD, 2):
                nc.scalar.dma_start(
                    xh_sb[0][:, k * 512:(k + 2) * 512],
                    xqbh[0, :, k * 512:(k + 2) * 512],
                )
                nc.scalar.dma_start(
                    xl_sb[0][:, k * 512:(k + 2) * 512],
                    xqbl[0, :, k * 512:(k + 2) * 512],
                )
            for g in range(1, NGH):
                xh_sb[g] = xpool.tile(
                    [P, KD * 512], bf16, name=f"xhs{g}", tag="xh"
                )
                nc.scalar.dma_start(xh_sb[g][:], xqbh[g, :, :])
                xl_sb[g] = xpool.tile(
                    [P, KD * 512], bf16, name=f"xls{g}", tag="xl"
                )
                nc.scalar.dma_start(xl_sb[g][:], xqbl[g, :, :])
            ident = cpool.tile([P, P], bf16)
            make_identity(nc, ident[:])
            amask_sb = cpool.tile([P, 4 * 512], bf16)
            nc.scalar.dma_start(amask_sb[:], amask4[:])
            tmask_sb = cpool.tile([P, P], bf16)
            nc.scalar.dma_start(tmask_sb[:], tmask[:])
            wv_sb = cpool.tile([P, KD * D], bf16)
            for k in range(KD):
                nc.scalar.dma_start(
                    wv_sb[:, k * D:(k + 1) * D], w_vT[k * P:(k + 1) * P, :]
                )

            # DRAM bounce buffers for the collectives (K in 2 chunks)
            kbounce = dpool.tile([4, HALF], bf16)
            kag = dpool.tile([8, HALF], bf16)
            vbounce = dpool.tile([HALF, D], bf16)
            vag = dpool.tile([T, D], bf16)

            # qk8 rows: [qh0,qh1, ql0,ql1, qh0,qh1, ql0,ql1]
            qk8 = qkpool.tile([8, HALF], bf16, tag="qk8")
            # kT8 rows: [kh0,kh1, kh0,kh1, kl0,kl1, kl0,kl1]; interleaved
            # supergroup layout: sg s cols [1024s,1024s+512)=even parity
            kT8 = qkpool.tile([8, T], bf16, tag="kT8")

            # PE warmup: junk matmuls so the HAM clock-gate reaches
            # full rate before the real phase-1 stream begins
            warm_ps = mmpsum.tile([P, 512], f32, space="PSUM", tag="mmps")
            for w in range(16):
                nc.tensor.matmul(
                    warm_ps[:],
                    lhsT=ident[:],
                    rhs=amask_sb[:, 0:512],
                    start=True,
                    stop=True,
                )
            for g in range(NGH):
                qk_ps = qkpsum.tile([4, 512], f32, space="PSUM", tag="qkps")
                for k in range(KD):
                    for t, (wc, xs) in enumerate(
                        ((0, xh_sb), (0, xl_sb), (4, xh_sb))
                    ):
                        nc.tensor.matmul(
                            qk_ps[:],
                            lhsT=wqk_sb[:, k * 8 + wc:k * 8 + wc + 4],
                            rhs=xs[g][:, k * 512:(k + 1) * 512],
                            start=(k == 0 and t == 0),
                            stop=(k == KD - 1 and t == 2),
                        )
                hi = ktpool.tile([4, 512], bf16, tag="hi")
                lo = ktpool.tile([4, 512], bf16, tag="lo")
                nc.vector.tensor_copy(hi[:], qk_ps[:])
                nc.vector.tensor_tensor(
                    out=lo[:], in0=qk_ps[:], in1=hi[:],
                    op=mybir.AluOpType.subtract,
                )
                cols = slice(g * 512, (g + 1) * 512)
                nc.sync.dma_start(qk8[0:2, cols], hi[0:2, :])
                nc.sync.dma_start(qk8[2:4, cols], lo[0:2, :])
                nc.sync.dma_start(qk8[4:6, cols], hi[0:2, :])
                nc.sync.dma_start(qk8[6:8, cols], lo[0:2, :])
                nc.sync.dma_start(
                    kbounce[0:2, g * 512:(g + 1) * 512], hi[2:4, :]
                )
                nc.sync.dma_start(
                    kbounce[2:4, g * 512:(g + 1) * 512], lo[2:4, :]
                )
            nc.gpsimd.collective_compute(
                "AllGather",
                mybir.AluOpType.bypass,
                replica_groups=GROUPS,
                ins=[kbounce[:].opt()],
                outs=[kag[:].opt()],
            )

            # assemble kT8 (even section from kag rows 0:4 = even core's
            # kh/kl; odd section from rows 4:8)
            kT8v = kT8[:].rearrange("r (s c) -> r s c", s=2 * NGH)
            for (dst0, src0) in ((0, 0), (2, 0), (4, 2), (6, 2)):
                # even sections: kT8[dst0:dst0+2, 1024s:1024s+512] for all s
                nc.sync.dma_start(
                    kT8v[dst0:dst0 + 2, 0:2 * NGH:2, :],
                    kag[src0:src0 + 2, :],
                )
                nc.sync.dma_start(
                    kT8v[dst0:dst0 + 2, 1:2 * NGH:2, :],
                    kag[src0 + 4:src0 + 6, :],
                )

            def v_tile(m):
                """V rows [m*128,(m+1)*128) = x_own tile @ W_V.T (bf16)."""
                g, mo = divmod(m, 4)
                vsb = vpool.tile([P, D], bf16)
                for n in range(2):
                    v_ps = vopsum.tile([P, 512], f32, space="PSUM", tag="vps")
                    for k in range(KD):
                        nc.tensor.matmul(
                            v_ps[:],
                            lhsT=xh_sb[g][:, k * 512 + mo * P:
                                          k * 512 + (mo + 1) * P],
                            rhs=wv_sb[:, k * D + n * 512:k * D + (n + 1) * 512],
                            start=(k == 0),
                            stop=(k == KD - 1),
                        )
                    nc.scalar.copy(vsb[:, n * 512:(n + 1) * 512], v_ps[:])
                nc.sync.dma_start(vbounce[m * P:(m + 1) * P, :], vsb[:])

            def v_ag_chunk(q):
                nc.gpsimd.collective_compute(
                    "AllGather",
                    mybir.AluOpType.bypass,
                    replica_groups=GROUPS,
                    ins=[vbounce[512 * q:512 * (q + 1), :].opt()],
                    outs=[vag[1024 * q:1024 * (q + 1), :].opt()],
                )

            v_emitted = 0
            v_ag_done = 0

            def pump_v(n):
                nonlocal v_emitted, v_ag_done
                for _ in range(n):
                    if v_emitted < MYT:
                        v_tile(v_emitted)
                        v_emitted += 1
                        if v_emitted % 4 == 0:
                            v_ag_chunk(v_ag_done)
                            v_ag_done += 1

            pump_v(2)

            # ---- phase 2: scores + argmax (V pumped between tiles) ----
            idx_tiles = []
            gathers_emitted = False

            def gather_tile(i):
                xg = xgpool.tile([P, D], bf16)
                nc.gpsimd.indirect_dma_start(
                    out=xg[:],
                    out_offset=None,
                    in_=vag[:],
                    in_offset=bass.IndirectOffsetOnAxis(
                        ap=idx_tiles[i][:, 0:1], axis=0
                    ),
                )
                nc.sync.dma_start(out[i, :, :], xg[:])

            for i in range(MYT):
                sstar, r = divmod(i, 4)
                W = 1024 * sstar + 512 + (r + 1) * P  # scanned width
                sc = scpool.tile([P, 2 * MYT * P], f32)

                for s in range(sstar + 1):
                    partial = s == sstar
                    # even section: cols [1024s, 1024s+512)
                    ps = mmpsum.tile([P, 512], f32, space="PSUM", tag="mmps")
                    if partial:
                        nc.tensor.matmul(
                            ps[:, r * P:512],
                            lhsT=ident[:],
                            rhs=amask_sb[:, 512 * r + P * r:512 * (r + 1)],
                            start=True,
                            stop=False,
                        )
                        nc.tensor.matmul(
                            ps[:, 0:(r + 1) * P],
                            lhsT=qk8[:, i * P:(i + 1) * P],
                            rhs=kT8[:, 1024 * s:1024 * s + (r + 1) * P],
                            start=False,
                            stop=True,
                        )
                    else:
                        nc.tensor.matmul(
                            ps[:],
                            lhsT=qk8[:, i * P:(i + 1) * P],
                            rhs=kT8[:, 1024 * s:1024 * s + 512],
                            start=True,
                            stop=True,
                        )
                    nc.scalar.copy(sc[:, 1024 * s:1024 * s + 512], ps[:])

                    # odd section: cols [1024s+512, 1024s+512+width)
                    width = (r + 1) * P if partial else 512
                    ps2 = mmpsum.tile([P, 512], f32, space="PSUM", tag="mmps")
                    if partial:
                        nc.tensor.matmul(
                            ps2[:, r * P:(r + 1) * P],
                            lhsT=ident[:],
                            rhs=tmask_sb[:],
                            start=True,
                            stop=False,
                        )
                    nc.tensor.matmul(
                        ps2[:, 0:width],
                        lhsT=qk8[:, i * P:(i + 1) * P],
                        rhs=kT8[:, 1024 * s + 512:1024 * s + 512 + width],
                        start=(not partial),
                        stop=True,
                    )
                    nc.scalar.copy(
                        sc[:, 1024 * s + 512:1024 * s + 512 + width],
                        ps2[:, 0:width],
                    )

                mx8 = spool.tile([P, 8], f32, tag="mx8")
                ix8 = idxpool.tile([P, 8], u32, tag="ix8")
                nc.vector.max(out=mx8[:], in_=sc[:, :W])
                nc.vector.max_index(out=ix8[:], in_max=mx8[:],
                                    in_values=sc[:, :W])
                idx_tiles.append(ix8)

                pump_v(2)

                if not gathers_emitted and v_ag_done == NGH:
                    gathers_emitted = True
                    for j in range(i + 1):
                        gather_tile(j)
                elif gathers_emitted:
                    gather_tile(i)

            while v_emitted < MYT:
                pump_v(1)
            if not gathers_emitted:
                for j in range(MYT):
                    gather_tile(j)

    nc.compile()
    return nc


def get_program():
    if "nc" not in _prog_cache:
        _prog_cache["nc"] = _build_program()
    return _prog_cache["nc"]


def make_core_inputs(x_full, W_Q, W_K, W_V):
    """Host-side shard: per-core input dicts."""
    import ml_dtypes

    bf16 = ml_dtypes.bfloat16
    x_full = np.ascontiguousarray(x_full, dtype=np.float32)
    w_qkT = np.concatenate([W_Q, W_K], axis=0).T.astype(np.float32)  # (D,4)
    wh = w_qkT.astype(bf16)
    wl = (w_qkT - wh.astype(np.float32)).astype(bf16)
    w_qk8 = np.ascontiguousarray(np.concatenate([wh, wl], axis=1))  # (D,8)
    w_vT = np.ascontiguousarray(np.asarray(W_V, np.float32).T.astype(bf16))

    r = np.arange(P)
    dmask = np.where(r[None, :] <= r[:, None], 0.0, NEG).astype(np.float32)
    zmask = np.zeros((P, P), np.float32)
    nmask = np.full((P, P), NEG, np.float32)

    in_maps = []
    tiles_per_core = []
    for c in range(N_CORES):
        b, h = divmod(c, 2)
        mine = [2 * i + h for i in range(MYT)]
        rows = np.concatenate(
            [np.arange(t * P, (t + 1) * P) for t in mine]
        )
        x_own = np.ascontiguousarray(x_full[b][rows])  # (2048, 1024)
        xqg = (
            x_own.reshape(NGH, 512, KD, P).transpose(0, 3, 2, 1)
            .reshape(NGH, P, KD * 512)
        )
        xqbh_a = xqg.astype(bf16)
        xqbl_a = (xqg - xqbh_a.astype(np.float32)).astype(bf16)

        am = dmask if h == 0 else zmask
        amask4 = np.zeros((P, 4 * 512), np.float32)
        for rr in range(4):
            blocks = [am] + [nmask] * (3 - rr)
            amask4[:, 512 * rr + P * rr:512 * (rr + 1)] = np.concatenate(
                blocks, axis=1
            )
        in_maps.append(
            {
                "xqbh": np.ascontiguousarray(xqbh_a),
                "xqbl": np.ascontiguousarray(xqbl_a),
                "w_qk8": w_qk8,
                "w_vT": w_vT,
                "amask4": np.ascontiguousarray(amask4.astype(bf16)),
                "tmask": np.ascontiguousarray(
                    (nmask if h == 0 else dmask).astype(bf16)
                ),
            }
        )
        tiles_per_core.append(mine)
    return in_maps, tiles_per_core


def assemble_output(results, tiles_per_core):
    out_full = np.empty((B, T, D), dtype=np.float32)
    for c in range(N_CORES):
        b = c // 2
        oc = np.asarray(results[c]["out"], dtype=np.float32)
        for i, th in enumerate(tiles_per_core[c]):
            out_full[b, th * P:(th + 1) * P, :] = oc[i]
    return out_full


def kernel(**inputs):
    from concourse.bass_utils import run_bass_kernel_spmd

    x_full = np.asarray(inputs["x"], dtype=np.float32)
    in_maps, tiles_per_core = make_core_inputs(
        x_full, np.asarray(inputs["W_Q"]), np.asarray(inputs["W_K"]),
        np.asarray(inputs["W_V"])
    )
    nc = get_program()
    try:
        res = run_bass_kernel_spmd(nc, in_maps, core_ids=list(range(N_CORES)))
    except Exception:
        # transient NRT device wedge: one retry
        res = run_bass_kernel_spmd(nc, in_maps, core_ids=list(range(N_CORES)))
    return assemble_output(res.results, tiles_per_core)
